# revision 5
# baseline (speedup 1.0000x reference)
"""Trainium2 Bass kernel for the FNO-style spectral layer.

Math: reference computes y = irfft(rfft(x) + delta) along L where delta
only touches output bins 0..63:
    delta[k] = fre[index[k]] * wr[k] + i * fim[index[k]] * wi[k]
By linearity of rfft/irfft, y = x + x @ P @ Q where
    P[n, k]      =  wr[k] * cos(2*pi*index[k]*n/L) / sqrt(L)
    P[n, 64+k]   = -wi[k] * sin(2*pi*index[k]*n/L) / sqrt(L)
    Q[k, n]      =  c_k * cos(2*pi*k*n/L)          (c_0 = 1/sqrt(L), else 2/sqrt(L))
    Q[64+k, n]   = -c_k * sin(2*pi*k*n/L)
(the jax irfft ignores the imaginary part of bin 0; row 64 of Q is zero
anyway since sin(0) == 0).

The norm rel-err budget (2e-2) is ~10x above bf16 I/O noise (~3e-3), so
everything runs in bf16: HBM traffic halves versus f32 (the kernel is
memory-bound — 45 MB/core, 90 MB per core-pair through one 716 GB/s
stack -> ~126 us floor vs ~255 us for f32).

The host uploads x PRE-TRANSPOSED per core as [NT, 125, 8, RB] where
element (t, p, c, r) = x_rows[t*RB + r, c*125 + p]. The device then
needs NO PE transposes at all:
    MM1: A^T[2m, RB] = sum_c P_c^T @ xt_c      (contract L in 8 chunks)
    MM2: corr^T chunk c = Q_c^T @ A^T          (per 125-row L chunk)
    y^T = x^T + corr^T   (elementwise adds split across DVE and GpSimd)
y is stored in the same transposed tiled layout and un-permuted on the
host. Per-partition DMA descriptors are 8 KB (loads/stores span all 8
chunks contiguously), well above the full-bandwidth knee.
"""

import sys

if "/opt/trn_rl_repo" not in sys.path:
    sys.path.insert(0, "/opt/trn_rl_repo")

import ml_dtypes
import numpy as np

import concourse.bass as bass  # noqa: F401  (kept for AP helpers)
import concourse.mybir as mybir
from concourse import bacc
from concourse.bass_utils import run_bass_kernel_spmd
from concourse.masks import make_identity
from concourse.tile import TileContext

B, E, L = 4096, 22, 1000
MODES = 64
M2 = 2 * MODES                # 128
NCORES = 8
ROWS = B * E                  # 90112
R_CORE = ROWS // NCORES       # 11264
RB = 512                      # batch-rows per tile
NT = R_CORE // RB             # 22
KC = 125                      # L-chunk (partition dim), 8 * 125 = 1000
NCH = L // KC                 # 8

F32 = mybir.dt.float32
BF16 = mybir.dt.bfloat16
NP_BF16 = ml_dtypes.bfloat16

# knobs (module-level so test.py can flip them before first kernel() call)
TRACE = False
LAST_RESULT = None


def _build_pq(fweights, fweights_im, index):
    """Host-side: analysis P [L, 2m] and synthesis Q [2m, L] in float64."""
    fw = np.asarray(fweights, dtype=np.float64)
    fwi = np.asarray(fweights_im, dtype=np.float64)
    idx = np.asarray(index, dtype=np.int64)
    m = idx.shape[0]
    widx = np.concatenate([[0], np.arange(1, m) + 1])
    wr = fw[widx, 0]
    wi = fwi[widx, 0]
    n = np.arange(L, dtype=np.float64)
    ang_in = 2.0 * np.pi * np.outer(n, idx.astype(np.float64)) / L
    P = np.zeros((L, 2 * m), dtype=np.float64)
    P[:, :m] = np.cos(ang_in) * wr / np.sqrt(L)
    P[:, m:] = -np.sin(ang_in) * wi / np.sqrt(L)
    k_out = np.arange(m, dtype=np.float64)
    ang_out = 2.0 * np.pi * np.outer(k_out, n) / L
    c = np.full(m, 2.0 / np.sqrt(L))
    c[0] = 1.0 / np.sqrt(L)
    Q = np.zeros((2 * m, L), dtype=np.float64)
    Q[:m, :] = np.cos(ang_out) * c[:, None]
    Q[m:, :] = -np.sin(ang_out) * c[:, None]
    return P, Q


_nc_cache = None


def _build_bass():
    nc = bacc.Bacc(None, target_bir_lowering=False)
    x_d = nc.dram_tensor("x", [NT, KC, NCH, RB], BF16, kind="ExternalInput")
    p_d = nc.dram_tensor("p", [KC, NCH, M2], BF16, kind="ExternalInput")
    q_d = nc.dram_tensor("q", [M2, NCH, KC], BF16, kind="ExternalInput")
    y_d = nc.dram_tensor("y", [NT, KC, NCH, RB], BF16, kind="ExternalOutput")

    with TileContext(nc) as tc:
        with (
            tc.tile_pool(name="consts", bufs=1) as consts,
            tc.tile_pool(name="xin", bufs=4) as xin,
            tc.tile_pool(name="apool", bufs=3) as apool,
            tc.tile_pool(name="yout", bufs=3) as yout,
            tc.tile_pool(name="ps_a", bufs=3, space="PSUM") as ps_a,
            tc.tile_pool(name="ps_c", bufs=4, space="PSUM") as ps_c,
        ):
            # params staged on the SWDGE (gpsimd) ring so the SP ring is
            # free for the first x loads
            pP = consts.tile([KC, NCH, M2], BF16)
            nc.gpsimd.dma_start(out=pP, in_=p_d[:, :, :])
            qQ = consts.tile([M2, NCH, KC], BF16)
            nc.gpsimd.dma_start(out=qQ, in_=q_d[:, :, :])
            ident = consts.tile([KC, KC], BF16)
            make_identity(nc, ident)

            for t in range(NT):
                x_sb = xin.tile([KC, NCH, RB], BF16, tag="x_sb")
                # early tiles load in quarters so MM1 starts sooner
                load_parts = (
                    [(0, 2), (2, 4), (4, 6), (6, 8)] if t <= 1 else [(0, NCH)]
                )
                for lo, hi in load_parts:
                    nc.sync.dma_start(
                        out=x_sb[:, lo:hi, :], in_=x_d[t, :, lo:hi, :]
                    )

                # MM1: A^T [2m, RB] accumulated over the 8 L-chunks
                a_ps = ps_a.tile([M2, RB], F32, tag="a_ps")
                for c in range(NCH):
                    nc.tensor.matmul(
                        a_ps,
                        pP[:, c, :],
                        x_sb[:, c, :],
                        start=(c == 0),
                        stop=(c == NCH - 1),
                    )
                a_sb = apool.tile([M2, RB], BF16, tag="a_sb")
                nc.scalar.copy(a_sb, a_ps)

                # MM2 + x-add per L-chunk. GPSIMD can't read PSUM, so the
                # add work is split: 5 chunks as DVE tensor_adds from
                # PSUM, 3 chunks fold x into the PSUM accumulation via an
                # identity matmul (PE has slack) leaving ACT a plain copy.
                y_sb = yout.tile([KC, NCH, RB], BF16, tag="y_sb")
                for c in range(NCH):
                    fold = c in (2, 5, 7)
                    ct_ps = ps_c.tile([KC, RB], F32, tag="ct_ps")
                    nc.tensor.matmul(
                        ct_ps, qQ[:, c, :], a_sb, start=True, stop=not fold
                    )
                    if fold:
                        nc.tensor.matmul(
                            ct_ps,
                            ident,
                            x_sb[:, c, :],
                            start=False,
                            stop=True,
                        )
                        nc.scalar.copy(y_sb[:, c, :], ct_ps)
                    else:
                        nc.vector.tensor_add(
                            y_sb[:, c, :], x_sb[:, c, :], ct_ps
                        )

                # stores on the ACT HWDGE ring so they never head-block
                # loads; late tiles store per-quarter to overlap drain
                store_parts = (
                    [(0, 2), (2, 4), (4, 6), (6, 8)]
                    if t >= NT - 2
                    else [(0, NCH)]
                )
                for lo, hi in store_parts:
                    nc.scalar.dma_start(
                        out=y_d[t, :, lo:hi, :], in_=y_sb[:, lo:hi, :]
                    )

    nc.compile()
    return nc


def kernel(x, fweights, fweights_im, index):
    global _nc_cache, LAST_RESULT
    x = np.asarray(x, dtype=np.float32)
    P, Q = _build_pq(fweights, fweights_im, index)
    p_host = np.ascontiguousarray(
        P.reshape(NCH, KC, M2).transpose(1, 0, 2)
    ).astype(NP_BF16)
    q_host = np.ascontiguousarray(Q.reshape(M2, NCH, KC)).astype(NP_BF16)

    if _nc_cache is None:
        _nc_cache = _build_bass()
    nc = _nc_cache

    xb = x.reshape(ROWS, L).astype(NP_BF16)
    in_maps = []
    for c in range(NCORES):
        xc = xb[c * R_CORE : (c + 1) * R_CORE]
        xt = np.ascontiguousarray(
            xc.reshape(NT, RB, NCH, KC).transpose(0, 3, 2, 1)
        )
        in_maps.append({"x": xt, "p": p_host, "q": q_host})

    res = run_bass_kernel_spmd(
        nc, in_maps, core_ids=list(range(NCORES)), trace=TRACE
    )
    LAST_RESULT = res
    y = np.empty((ROWS, L), dtype=np.float32)
    for c in range(NCORES):
        yt = res.results[c]["y"]  # [NT, KC, NCH, RB] bf16
        y[c * R_CORE : (c + 1) * R_CORE] = (
            yt.transpose(0, 3, 2, 1).reshape(R_CORE, L).astype(np.float32)
        )
    return y.reshape(B, 1, E, L)
